# Initial kernel scaffold
#
"""Trainium2 Bass kernel for nn_Attn2d (3x3 local window attention, 8 heads).

Sharding: 8 cores = (batch 4) x (H halves 2). Each core gets a halo-extended
slice of x (34 rows incl 1-row halo each side, zero-filled outside the image),
computes the 1x1 conv projection + windowed attention for its 32 own rows.

Per-core layout: channel-major [ch, px] with a width-padded pixel axis
(34 rows x 68 cols, interior at cols 2..65, pads zeroed) so the 9 window
shifts are plain free-axis offset views.

Math per pixel p, head n (d=dv=32):
  qkv = W x + b  (scale folded into W_q, W_k rows on host)
  logit[n,dl] = sum_d q[nd,p] * k[nd, p+dl] + sum_d q[nd,p] * pos[nd, dl]
  attn = softmax_dl ; o[c,p] = sum_dl attn[n(c),dl] * v[c, p+dl]

Engine split: PE = projection (fp32r) + pos-term + blockdiag channel
reductions + head->channel expands (bf16 0/1 matrices, DMA'd pre-cast from
host); the 9-shift q*k and attn*v products are spread across DVE (2x bf16
mode where views are 4-byte aligned), GPSIMD (alignment-insensitive), and
ACT (psum->bf16 casts feeding GPSIMD); softmax is normalized in compact
[72, px] space and hoisted into the logits stage; chunks are software-
pipelined so logits(ci+1) overlaps AV(ci).
"""
import numpy as np

import concourse.mybir as mybir
import concourse.tile as tile
from concourse import bacc

F32 = mybir.dt.float32
F32R = mybir.dt.float32r
BF16 = mybir.dt.bfloat16
AF = mybir.ActivationFunctionType

# problem constants (hardcoded per contract)
B, CIN, H, W = 4, 256, 64, 64
QK = 256
OUT = 256
NH = 8
KW = 3
D = QK // NH          # 32
SCALE = float(D) ** (-0.25)
NCORES = 8

HOWN = H // 2         # 32 own rows per core
HS = HOWN + 2         # 34 rows incl halo
WP = W + 4            # 68 padded width (interior cols 2..65)
C0 = 2                # first interior column
PXP = HS * WP         # padded pixels
PXU = HS * W          # 2176 unpadded input pixels
OWNPX = HOWN * W      # 2048 own pixels
NKK = KW * KW         # 9
NL = NH * NKK         # 72 (n, delta) pairs

CHUNK = 512           # px per attention chunk (= 8 rows)
NCHUNK = OWNPX // CHUNK


def _build_nc():
    nc = bacc.Bacc()

    xin = nc.declare_dram_parameter("x", [CIN, PXU], F32, isOutput=False)
    wt = nc.declare_dram_parameter("wt", [CIN, 3 * QK], F32, isOutput=False)
    biasd = nc.declare_dram_parameter("bias", [128, 6], F32, isOutput=False)
    posd = nc.declare_dram_parameter("posblk", [CIN, NL], BF16, isOutput=False)
    redd = nc.declare_dram_parameter("redmat", [CIN, NKK * NL], BF16, isOutput=False)
    sum9d = nc.declare_dram_parameter("sum9", [NL, NH], BF16, isOutput=False)
    e8d = nc.declare_dram_parameter("e8", [NH, NL], BF16, isOutput=False)
    expd = nc.declare_dram_parameter("expall", [NL, NKK * CIN], BF16, isOutput=False)
    identd = nc.declare_dram_parameter("ident", [128, 128], BF16, isOutput=False)
    edged = nc.declare_dram_parameter("edge", [128, 2], F32, isOutput=False)
    outd = nc.declare_dram_parameter("o", [OUT, OWNPX], F32, isOutput=True)

    with tile.TileContext(nc) as tc:
        with (
            tc.tile_pool(name="const", bufs=1) as constp,
            tc.tile_pool(name="data", bufs=1) as datap,
            tc.tile_pool(name="work", bufs=4) as workp,
            tc.tile_pool(name="psl", bufs=1, space="PSUM") as psl,
            tc.tile_pool(name="pse", bufs=2, space="PSUM") as pse,
            tc.tile_pool(name="pso", bufs=1, space="PSUM") as pso,
        ):
            # ---- load inputs ----
            x_t = [datap.tile([128, PXU], F32, tag=f"x{t}", name=f"x{t}") for t in range(2)]
            wt_t = [datap.tile([128, 3 * QK], F32, tag=f"wt{t}", name=f"wt{t}") for t in range(2)]
            XQ = PXU // 4
            for t in range(2):
                nc.sync.dma_start(wt_t[t][:], wt[t * 128:(t + 1) * 128, :])
            for qi in range(4):
                for t in range(2):
                    eng = nc.sync if (qi + t) % 2 == 0 else nc.gpsimd
                    eng.dma_start(x_t[t][:, qi * XQ:(qi + 1) * XQ],
                                  xin[t * 128:(t + 1) * 128, qi * XQ:(qi + 1) * XQ])
            bias_t = constp.tile([128, 6], F32, tag="bias", name="bias")
            nc.gpsimd.dma_start(bias_t[:], biasd[:])
            pos_r = [constp.tile([128, NL], BF16, tag=f"posr{t}", name=f"posr{t}") for t in range(2)]
            for t in range(2):
                nc.gpsimd.dma_start(pos_r[t][:], posd[t * 128:(t + 1) * 128, :])
            red_r = [constp.tile([128, NKK * NL], BF16, tag=f"redr{t}", name=f"redr{t}") for t in range(2)]
            for t in range(2):
                nc.gpsimd.dma_start(red_r[t][:], redd[t * 128:(t + 1) * 128, :])
            sum9_r = constp.tile([NL, NH], BF16, tag="sum9r", name="sum9r")
            nc.gpsimd.dma_start(sum9_r[:], sum9d[:])
            e8_r = constp.tile([NH, NL], BF16, tag="e8r", name="e8r")
            nc.gpsimd.dma_start(e8_r[:], e8d[:])
            exp_r = constp.tile([NL, NKK * CIN], BF16, tag="expallr", name="expallr")
            nc.gpsimd.dma_start(exp_r[:], expd[:])
            ident_r = constp.tile([128, 128], BF16, tag="identr", name="identr")
            nc.gpsimd.dma_start(ident_r[:], identd[:])
            edge_t = constp.tile([128, 2], F32, tag="edge", name="edge")
            nc.gpsimd.dma_start(edge_t[:], edged[:])

            # ---- round matmul operands: fp32r for proj, bf16 for attention ----
            x_r = [datap.tile([128, PXU], F32R, tag=f"xr{t}", name=f"xr{t}") for t in range(2)]
            wt_r = [datap.tile([128, 3 * QK], F32R, tag=f"wtr{t}", name=f"wtr{t}") for t in range(2)]
            for t in range(2):
                nc.vector.tensor_copy(wt_r[t][:], wt_t[t][:])
            for qi in range(4):
                for t in range(2):
                    nc.vector.tensor_copy(x_r[t][:, qi * XQ:(qi + 1) * XQ],
                                          x_t[t][:, qi * XQ:(qi + 1) * XQ])

            # ---- q/k/v bf16 tiles, padded layout [128, 34*68] ----
            # k/v have base + odd (shifted-by-one: ko[c] = k[c+1]) copies so
            # every window shift reads 4-byte aligned (DVE 2x bf16 mode).
            def mktiles(prefix):
                return [datap.tile([128, PXP], BF16, tag=f"{prefix}{t}", name=f"{prefix}{t}")
                        for t in range(2)]

            q_b = mktiles("qb")
            k_b = mktiles("kb")
            v_b = [datap.tile([128, PXP], F32R, tag=f"vb{t}", name=f"vb{t}")
                   for t in range(2)]
            # zero only the pad columns (proj overwrites interior after);
            # f32-bitcast col j covers bf16 cols 2j,2j+1
            for t in range(2):
                # bf16 k: f32-bitcast halves the element count (col j = bf16 2j,2j+1)
                fk = k_b[t][:].bitcast(F32).rearrange("p (r c) -> p r c", c=WP // 2)
                nc.gpsimd.memset(fk[:, :, 0:2], 0.0)
                nc.gpsimd.memset(fk[:, :, WP // 2 - 2:WP // 2], 0.0)
                # f32r v: 4-byte, bitcast keeps the 68-wide row structure
                fv = v_b[t][:].bitcast(F32).rearrange("p (r c) -> p r c", c=WP)
                nc.gpsimd.memset(fv[:, :, 0:2], 0.0)
                nc.gpsimd.memset(fv[:, :, WP - 2:WP], 0.0)

            # ---- projection ----
            pxc = [448, 448, 448, 448, 384]
            pxo = [0, 448, 896, 1344, 1792]

            def pad_view(tl, r0, nr, c0, cw=W):
                return tl[:].rearrange("p (r c) -> p r c", c=WP)[:, r0:r0 + nr, c0:c0 + cw]

            for m in range(6):
                grp = m // 2        # 0=q, 1=k, 2=v
                t = m % 2
                for ci in range(5):
                    cw, co = pxc[ci], pxo[ci]
                    pp = pse.tile([128, cw], F32, tag="pe", name="pp", bufs=3)
                    for kt in range(2):
                        nc.tensor.matmul(
                            pp[:],
                            wt_r[kt][:, m * 128:(m + 1) * 128],
                            x_r[kt][:, co:co + cw],
                            start=(kt == 0),
                            stop=(kt == 1),
                        )
                    r0, nr = co // W, cw // W
                    if grp == 0:
                        nc.scalar.activation(pad_view(q_b[t], r0, nr, C0), pp[:],
                                             AF.Identity, bias=bias_t[:, m:m + 1])
                    elif grp == 1:
                        nc.scalar.activation(pad_view(k_b[t], r0, nr, C0), pp[:],
                                             AF.Identity, bias=bias_t[:, m:m + 1])

                    else:
                        nc.vector.tensor_scalar_add(pad_view(v_b[t], r0, nr, C0), pp[:],
                                                    bias_t[:, m:m + 1])

            # f32r twins of the expand-side 0/1 matrices so post-softmax
            # matmuls pair f32r x f32r; emitted after proj so the ACT copies
            # fill the logits-phase slack instead of delaying proj
            expall_r2 = constp.tile([NL, NKK * CIN], F32R, tag="expallr2", name="expallr2")
            nc.scalar.copy(expall_r2[:], exp_r[:])
            ident_r2 = constp.tile([128, 128], F32R, tag="identr2", name="identr2")
            nc.scalar.copy(ident_r2[:], ident_r[:])

            # ---- edge mask: zero k/v halo rows that fall outside the image ----
            for tl in (k_b, v_b):
                for t in range(2):
                    pv = tl[t][:].rearrange("p (r c) -> p r c", c=WP)
                    nc.gpsimd.tensor_scalar_mul(pv[:, 0, :], pv[:, 0, :], edge_t[:, 0:1])
                    nc.gpsimd.tensor_scalar_mul(pv[:, HS - 1, :], pv[:, HS - 1, :], edge_t[:, 1:2])

            # ---- attention over 4 chunks of 8 rows ----
            def own_view(tl, ci):
                return pad_view(tl, 1 + 8 * ci, 8, C0)

            def emit_logits(ci):
                # logits [72, 512] = qpos + sum_d q*k_shift reductions, then exp
                pl = psl.tile([NL, CHUNK], F32, tag="pl", name="pl", bufs=2)
                for t in range(2):
                    nc.tensor.matmul(
                        pl[:], pos_r[t][:], own_view(q_b[t], ci),
                        start=(t == 0), stop=False, skip_group_check=True,
                    )
                for dl in range(NKK):
                    di, dj = dl // KW, dl % KW
                    for t in range(2):
                        pr = workp.tile([128, CHUNK], BF16, tag=f"prod{t}", name=f"prod{t}", bufs=8)
                        eng = nc.vector if dj == 1 else nc.gpsimd
                        kv = pad_view(k_b[t], 8 * ci + di, 8, dj + 1)
                        eng.tensor_mul(pr[:], own_view(q_b[t], ci), kv)
                        nc.tensor.matmul(
                            pl[:], red_r[t][:, dl * NL:(dl + 1) * NL], pr[:],
                            start=False, stop=(dl == NKK - 1 and t == 1),
                            skip_group_check=True,
                        )
                e_t = workp.tile([NL, CHUNK], BF16, tag="e", name="e", bufs=3)
                nc.scalar.activation(e_t[:], pl[:], AF.Exp)
                # normalize in compact [72, px] space right here so the AV
                # stage can start with expand matmuls immediately
                pz = pse.tile([NH, CHUNK], F32, tag="pzx", name="pz", bufs=1)
                nc.tensor.matmul(pz[:], sum9_r[:], e_t[:],
                                 start=True, stop=True, skip_group_check=True)
                rz8 = workp.tile([NH, CHUNK], BF16, tag="rz8", name="rz8")
                with nc.allow_low_precision(reason="bf16 softmax denominators"):
                    nc.vector.reciprocal(rz8[:], pz[:])
                pzr = pse.tile([NL, CHUNK], F32, tag="pzx", name="pzr", bufs=1)
                nc.tensor.matmul(pzr[:], e8_r[:], rz8[:],
                                 start=True, stop=True, skip_group_check=True)
                attn = workp.tile([NL, CHUNK], F32R, tag="attn", name="attn", bufs=3)
                nc.vector.tensor_mul(attn[:], e_t[:], pzr[:])
                return attn

            def emit_av(ci, attn):
                # AV: po[t] = sum_dl expand(attn_dl) * v_shift  (final output)
                po = [pso.tile([128, CHUNK], F32, tag=f"po{t}", name=f"po{t}") for t in range(2)]
                for dl in range(NKK):
                    di, dj = dl // KW, dl % KW
                    for t in range(2):
                        pe = pse.tile([128, CHUNK], F32, tag="pe", name="pe", bufs=3)
                        nc.tensor.matmul(
                            pe[:],
                            expall_r2[:, dl * CIN + t * 128: dl * CIN + (t + 1) * 128],
                            attn[:], start=True, stop=True, skip_group_check=True,
                        )
                        vv = pad_view(v_b[t], 8 * ci + di, 8, dj + 1)
                        if dl % 2 == 0:
                            # DVE reads psum directly (mixed-dtype 1x), f32r out
                            p2 = workp.tile([128, CHUNK], F32R, tag=f"p2r{t}", name=f"p2r{t}", bufs=5)
                            nc.vector.tensor_mul(p2[:], pe[:], vv)
                            acc_ident = ident_r2
                        else:
                            # ACT casts, gpsimd multiplies (both idle in this phase)
                            eb = workp.tile([128, CHUNK], F32R, tag=f"eb{t}", name=f"eb{t}", bufs=3)
                            nc.scalar.copy(eb[:], pe[:])
                            p2 = workp.tile([128, CHUNK], BF16, tag=f"p2{t}", name=f"p2{t}", bufs=5)
                            nc.gpsimd.tensor_mul(p2[:], eb[:], vv)
                            acc_ident = ident_r
                        nc.tensor.matmul(
                            po[t][:], acc_ident[:], p2[:],
                            start=(dl == 0), stop=(dl == NKK - 1),
                            skip_group_check=True,
                        )
                for t in range(2):
                    ob = workp.tile([128, CHUNK], F32, tag=f"ob{t}", name=f"ob{t}")
                    nc.scalar.copy(ob[:], po[t][:])
                    nc.sync.dma_start(
                        outd[t * 128:(t + 1) * 128, ci * CHUNK:(ci + 1) * CHUNK], ob[:]
                    )

            # software pipeline: logits+softmax(ci) overlaps AV(ci-1)
            attn_prev = emit_logits(0)
            for ci in range(1, NCHUNK):
                attn_cur = emit_logits(ci)
                emit_av(ci - 1, attn_prev)
                attn_prev = attn_cur
            emit_av(NCHUNK - 1, attn_prev)

    nc.finalize()
    return nc


_CACHE = {}


def _host_consts(w_proj, b_proj, pos_feats):
    wT = np.ascontiguousarray(w_proj.T).astype(np.float32).copy()   # [256, 768]
    wT[:, : 2 * QK] *= SCALE
    b = np.asarray(b_proj, np.float32).copy()
    b[: 2 * QK] *= SCALE
    bias = np.ascontiguousarray(b.reshape(6, 128).T)                # [128, 6]

    heads = np.arange(CIN) // D                                     # head of channel
    posblk = np.zeros((CIN, NL), np.float32)
    for c in range(CIN):
        n = heads[c]
        posblk[c, n * NKK:(n + 1) * NKK] = pos_feats[c]

    redmat = np.zeros((CIN, NKK * NL), np.float32)
    for dl in range(NKK):
        for c in range(CIN):
            redmat[c, dl * NL + heads[c] * NKK + dl] = 1.0

    sum9 = np.zeros((NL, NH), np.float32)
    e8 = np.zeros((NH, NL), np.float32)
    for n in range(NH):
        for dl in range(NKK):
            sum9[n * NKK + dl, n] = 1.0
            e8[n, n * NKK + dl] = 1.0

    expall = np.zeros((NL, NKK * CIN), np.float32)
    for dl in range(NKK):
        for n in range(NH):
            expall[n * NKK + dl, dl * CIN + n * D: dl * CIN + (n + 1) * D] = 1.0

    import ml_dtypes
    bf = ml_dtypes.bfloat16
    ident = np.eye(128, dtype=np.float32)
    return (wT, bias, posblk.astype(bf), redmat.astype(bf), sum9.astype(bf),
            e8.astype(bf), expall.astype(bf), ident.astype(bf))


def make_in_maps(x, w_proj, b_proj, pos_feats):
    x = np.asarray(x, np.float32)
    wT, bias, posblk, redmat, sum9, e8, expall, ident = _host_consts(
        np.asarray(w_proj, np.float32),
        np.asarray(b_proj, np.float32),
        np.asarray(pos_feats, np.float32),
    )
    in_maps = []
    for s in range(NCORES):
        b_i, half = s // 2, s % 2
        xs = np.zeros((CIN, HS, W), np.float32)
        h0 = half * HOWN - 1                  # global row of local row 0
        lo, hi = max(h0, 0), min(h0 + HS, H)
        xs[:, lo - h0:hi - h0, :] = x[b_i, :, lo:hi, :]
        edge = np.ones((128, 2), np.float32)
        if half == 0:
            edge[:, 0] = 0.0
        if half == 1:
            edge[:, 1] = 0.0
        in_maps.append({
            "x": np.ascontiguousarray(xs.reshape(CIN, PXU)),
            "wt": wT, "bias": bias, "posblk": posblk, "redmat": redmat,
            "sum9": sum9, "e8": e8, "expall": expall, "ident": ident, "edge": edge,
        })
    return in_maps


def kernel(x, w_proj, b_proj, pos_feats):
    from concourse.bass_utils import run_bass_kernel_spmd

    if "nc" not in _CACHE:
        _CACHE["nc"] = _build_nc()
    nc = _CACHE["nc"]
    in_maps = make_in_maps(x, w_proj, b_proj, pos_feats)
    res = run_bass_kernel_spmd(nc, in_maps, list(range(NCORES)))
    out = np.zeros((B, OUT, H, W), np.float32)
    for s in range(NCORES):
        b_i, half = s // 2, s % 2
        out[b_i, :, half * HOWN:(half + 1) * HOWN, :] = (
            res.results[s]["o"].reshape(OUT, HOWN, W)
        )
    return out



# revision 3
# speedup vs baseline: 1.0873x; 1.0873x over previous
"""Trainium2 Bass kernel for nn_Attn2d (3x3 local window attention, 8 heads).

Sharding: 8 cores = (batch 4) x (H halves 2). Each core gets a halo-extended
slice of x (34 rows incl 1-row halo each side, zero-filled outside the image),
computes the 1x1 conv projection + windowed attention for its 32 own rows.

Per-core layout: channel-major [ch, px] with a width-padded pixel axis
(34 rows x 68 cols, interior at cols 2..65, pads zeroed) so the 9 window
shifts are plain free-axis offset views.

Math per pixel p, head n (d=dv=32):
  qkv = W x + b  (scale folded into W_q, W_k rows on host)
  logit[n,dl] = sum_d q[nd,p] * k[nd, p+dl] + sum_d q[nd,p] * pos[nd, dl]
  attn = softmax_dl ; o[c,p] = sum_dl attn[n(c),dl] * v[c, p+dl]

Engine split: PE = projection (fp32r) + pos-term + blockdiag channel
reductions + head->channel expands (bf16 0/1 matrices, DMA'd pre-cast from
host); the 9-shift q*k and attn*v products are spread across DVE (2x bf16
mode where views are 4-byte aligned), GPSIMD (alignment-insensitive), and
ACT (psum->bf16 casts feeding GPSIMD); softmax is normalized in compact
[72, px] space and hoisted into the logits stage; chunks are software-
pipelined so logits(ci+1) overlaps AV(ci).
"""
import numpy as np

import concourse.mybir as mybir
import concourse.tile as tile
from concourse import bacc

F32 = mybir.dt.float32
F32R = mybir.dt.float32r
BF16 = mybir.dt.bfloat16
AF = mybir.ActivationFunctionType

# problem constants (hardcoded per contract)
B, CIN, H, W = 4, 256, 64, 64
QK = 256
OUT = 256
NH = 8
KW = 3
D = QK // NH          # 32
SCALE = float(D) ** (-0.25)
NCORES = 8

HOWN = H // 2         # 32 own rows per core
HS = HOWN + 2         # 34 rows incl halo
WP = W + 4            # 68 padded width (interior cols 2..65)
C0 = 2                # first interior column
PXP = HS * WP         # padded pixels
PXU = HS * W          # 2176 unpadded input pixels
OWNPX = HOWN * W      # 2048 own pixels
NKK = KW * KW         # 9
NL = NH * NKK         # 72 (n, delta) pairs

CHUNK = 512           # px per attention chunk (= 8 rows)
NCHUNK = OWNPX // CHUNK


def _build_nc():
    nc = bacc.Bacc()

    xin = nc.declare_dram_parameter("x", [CIN, PXU], F32, isOutput=False)
    wt = nc.declare_dram_parameter("wt", [CIN, 3 * QK], F32, isOutput=False)
    biasd = nc.declare_dram_parameter("bias", [128, 6], F32, isOutput=False)
    posd = nc.declare_dram_parameter("posblk", [CIN, NL], BF16, isOutput=False)
    redd = nc.declare_dram_parameter("redmat", [CIN, NKK * NL], BF16, isOutput=False)
    sum9d = nc.declare_dram_parameter("sum9", [NL, NH], BF16, isOutput=False)
    e8d = nc.declare_dram_parameter("e8", [NH, NL], BF16, isOutput=False)
    expd = nc.declare_dram_parameter("expall", [NL, NKK * CIN], BF16, isOutput=False)
    identd = nc.declare_dram_parameter("ident", [128, 128], BF16, isOutput=False)
    edged = nc.declare_dram_parameter("edge", [128, 2], F32, isOutput=False)
    outd = nc.declare_dram_parameter("o", [OUT, OWNPX], F32, isOutput=True)

    with tile.TileContext(nc) as tc:
        with (
            tc.tile_pool(name="const", bufs=1) as constp,
            tc.tile_pool(name="data", bufs=1) as datap,
            tc.tile_pool(name="work", bufs=4) as workp,
            tc.tile_pool(name="psl", bufs=1, space="PSUM") as psl,
            tc.tile_pool(name="pse", bufs=2, space="PSUM") as pse,
            tc.tile_pool(name="pso", bufs=1, space="PSUM") as pso,
        ):
            # ---- PE warmup: start the p-state ramp during the DMA phase ----
            wz = constp.tile([1, 2], BF16, tag="wz", name="wz")
            nc.gpsimd.memset(wz[:], 0.0)
            pwz = pse.tile([1, 2], F32, tag="pe", name="pwz", bufs=3)
            nc.tensor.matmul(pwz[:, 0:1], wz[:, 0:1], wz[:, 1:2],
                             start=True, stop=True, skip_group_check=True)

            # ---- load inputs ----
            x_t = [datap.tile([128, PXU], F32, tag=f"x{t}", name=f"x{t}") for t in range(2)]
            wt_t = [datap.tile([128, 3 * QK], F32, tag=f"wt{t}", name=f"wt{t}") for t in range(2)]
            XQ = PXU // 4
            for t in range(2):
                nc.sync.dma_start(wt_t[t][:], wt[t * 128:(t + 1) * 128, :])
            xq = [nc.sync, nc.gpsimd, nc.scalar]
            for qi in range(4):
                for t in range(2):
                    eng = xq[(2 * qi + t) % 3]
                    eng.dma_start(x_t[t][:, qi * XQ:(qi + 1) * XQ],
                                  xin[t * 128:(t + 1) * 128, qi * XQ:(qi + 1) * XQ])
            bias_t = constp.tile([128, 6], F32, tag="bias", name="bias")
            nc.gpsimd.dma_start(bias_t[:], biasd[:])
            pos_r = [constp.tile([128, NL], BF16, tag=f"posr{t}", name=f"posr{t}") for t in range(2)]
            for t in range(2):
                nc.gpsimd.dma_start(pos_r[t][:], posd[t * 128:(t + 1) * 128, :])
            red_r = [constp.tile([128, NKK * NL], BF16, tag=f"redr{t}", name=f"redr{t}") for t in range(2)]
            for t in range(2):
                nc.gpsimd.dma_start(red_r[t][:], redd[t * 128:(t + 1) * 128, :])
            edge_t = constp.tile([128, 2], F32, tag="edge", name="edge")
            nc.gpsimd.dma_start(edge_t[:], edged[:])
            sum9_r = constp.tile([NL, NH], BF16, tag="sum9r", name="sum9r")
            nc.scalar.dma_start(sum9_r[:], sum9d[:])
            e8_r = constp.tile([NH, NL], BF16, tag="e8r", name="e8r")
            nc.scalar.dma_start(e8_r[:], e8d[:])
            exp_r = constp.tile([NL, NKK * CIN], BF16, tag="expallr", name="expallr")
            nc.scalar.dma_start(exp_r[:], expd[:])
            ident_r = constp.tile([128, 128], BF16, tag="identr", name="identr")
            nc.scalar.dma_start(ident_r[:], identd[:])

            # ---- round matmul operands: fp32r for proj, bf16 for attention ----
            x_r = [datap.tile([128, PXU], F32R, tag=f"xr{t}", name=f"xr{t}") for t in range(2)]
            wt_r = [datap.tile([128, 3 * QK], F32R, tag=f"wtr{t}", name=f"wtr{t}") for t in range(2)]
            for t in range(2):
                nc.vector.tensor_copy(wt_r[t][:], wt_t[t][:])
            for qi in range(4):
                for t in range(2):
                    nc.vector.tensor_copy(x_r[t][:, qi * XQ:(qi + 1) * XQ],
                                          x_t[t][:, qi * XQ:(qi + 1) * XQ])

            # ---- q/k/v bf16 tiles, padded layout [128, 34*68] ----
            # k/v have base + odd (shifted-by-one: ko[c] = k[c+1]) copies so
            # every window shift reads 4-byte aligned (DVE 2x bf16 mode).
            def mktiles(prefix):
                return [datap.tile([128, PXP], BF16, tag=f"{prefix}{t}", name=f"{prefix}{t}")
                        for t in range(2)]

            q_b = mktiles("qb")
            k_b = mktiles("kb")
            v_b = [datap.tile([128, PXP], F32R, tag=f"vb{t}", name=f"vb{t}")
                   for t in range(2)]
            # zero only the pad columns (proj overwrites interior after);
            # f32-bitcast col j covers bf16 cols 2j,2j+1
            for t in range(2):
                # bf16 k: f32-bitcast halves the element count (col j = bf16 2j,2j+1)
                fk = k_b[t][:].bitcast(F32).rearrange("p (r c) -> p r c", c=WP // 2)
                nc.gpsimd.memset(fk[:, :, 0:2], 0.0)
                nc.gpsimd.memset(fk[:, :, WP // 2 - 2:WP // 2], 0.0)
                # f32r v: 4-byte, bitcast keeps the 68-wide row structure
                fv = v_b[t][:].bitcast(F32).rearrange("p (r c) -> p r c", c=WP)
                nc.gpsimd.memset(fv[:, :, 0:2], 0.0)
                nc.gpsimd.memset(fv[:, :, WP - 2:WP], 0.0)

            # ---- projection ----
            pxc = [448, 448, 448, 448, 384]
            pxo = [0, 448, 896, 1344, 1792]

            def pad_view(tl, r0, nr, c0, cw=W):
                return tl[:].rearrange("p (r c) -> p r c", c=WP)[:, r0:r0 + nr, c0:c0 + cw]

            for m in range(6):
                grp = m // 2        # 0=q, 1=k, 2=v
                t = m % 2
                for ci in range(5):
                    cw, co = pxc[ci], pxo[ci]
                    pp = pse.tile([128, cw], F32, tag="pe", name="pp", bufs=3)
                    for kt in range(2):
                        nc.tensor.matmul(
                            pp[:],
                            wt_r[kt][:, m * 128:(m + 1) * 128],
                            x_r[kt][:, co:co + cw],
                            start=(kt == 0),
                            stop=(kt == 1),
                        )
                    r0, nr = co // W, cw // W
                    if grp == 0:
                        nc.scalar.activation(pad_view(q_b[t], r0, nr, C0), pp[:],
                                             AF.Identity, bias=bias_t[:, m:m + 1])
                    elif grp == 1:
                        nc.scalar.activation(pad_view(k_b[t], r0, nr, C0), pp[:],
                                             AF.Identity, bias=bias_t[:, m:m + 1])

                    else:
                        nc.vector.tensor_scalar_add(pad_view(v_b[t], r0, nr, C0), pp[:],
                                                    bias_t[:, m:m + 1])

            # f32r twins of the expand-side 0/1 matrices so post-softmax
            # matmuls pair f32r x f32r; emitted after proj so the ACT copies
            # fill the logits-phase slack instead of delaying proj
            expall_r2 = constp.tile([NL, NKK * CIN], F32R, tag="expallr2", name="expallr2")
            nc.scalar.copy(expall_r2[:], exp_r[:])
            ident_r2 = constp.tile([128, 128], F32R, tag="identr2", name="identr2")
            nc.scalar.copy(ident_r2[:], ident_r[:])

            # ---- edge mask: zero k/v halo rows that fall outside the image ----
            for tl in (k_b, v_b):
                for t in range(2):
                    pv = tl[t][:].rearrange("p (r c) -> p r c", c=WP)
                    nc.gpsimd.tensor_scalar_mul(pv[:, 0, :], pv[:, 0, :], edge_t[:, 0:1])
                    nc.gpsimd.tensor_scalar_mul(pv[:, HS - 1, :], pv[:, HS - 1, :], edge_t[:, 1:2])

            # ---- attention over 4 chunks of 8 rows ----
            def own_view(tl, ci):
                return pad_view(tl, 1 + 8 * ci, 8, C0)

            def emit_logits(ci):
                # logits [72, 512] = qpos + sum_d q*k_shift reductions, then exp
                pl = psl.tile([NL, CHUNK], F32, tag="pl", name="pl", bufs=2)
                for t in range(2):
                    nc.tensor.matmul(
                        pl[:], pos_r[t][:], own_view(q_b[t], ci),
                        start=(t == 0), stop=False, skip_group_check=True,
                    )
                for dl in range(NKK):
                    di, dj = dl // KW, dl % KW
                    for t in range(2):
                        pr = workp.tile([128, CHUNK], BF16, tag=f"prod{t}", name=f"prod{t}", bufs=8)
                        eng = nc.vector if dj == 1 else nc.gpsimd
                        kv = pad_view(k_b[t], 8 * ci + di, 8, dj + 1)
                        eng.tensor_mul(pr[:], own_view(q_b[t], ci), kv)
                        nc.tensor.matmul(
                            pl[:], red_r[t][:, dl * NL:(dl + 1) * NL], pr[:],
                            start=False, stop=(dl == NKK - 1 and t == 1),
                            skip_group_check=True,
                        )
                e_t = workp.tile([NL, CHUNK], BF16, tag="e", name="e", bufs=3)
                nc.scalar.activation(e_t[:], pl[:], AF.Exp)
                # normalize in compact [72, px] space right here so the AV
                # stage can start with expand matmuls immediately
                pz = pse.tile([NH, CHUNK], F32, tag="pzx", name="pz", bufs=1)
                nc.tensor.matmul(pz[:], sum9_r[:], e_t[:],
                                 start=True, stop=True, skip_group_check=True)
                rz8 = workp.tile([NH, CHUNK], BF16, tag="rz8", name="rz8")
                with nc.allow_low_precision(reason="bf16 softmax denominators"):
                    nc.vector.reciprocal(rz8[:], pz[:])
                pzr = pse.tile([NL, CHUNK], F32, tag="pzx", name="pzr", bufs=1)
                nc.tensor.matmul(pzr[:], e8_r[:], rz8[:],
                                 start=True, stop=True, skip_group_check=True)
                attn = workp.tile([NL, CHUNK], F32R, tag="attn", name="attn", bufs=3)
                nc.vector.tensor_mul(attn[:], e_t[:], pzr[:])
                return attn

            def emit_av(ci, attn):
                # AV: po[t] = sum_dl expand(attn_dl) * v_shift  (final output)
                po = [pso.tile([128, CHUNK], F32, tag=f"po{t}", name=f"po{t}") for t in range(2)]
                for dl in range(NKK):
                    di, dj = dl // KW, dl % KW
                    for t in range(2):
                        pe = pse.tile([128, CHUNK], F32, tag="pe", name="pe", bufs=3)
                        nc.tensor.matmul(
                            pe[:],
                            expall_r2[:, dl * CIN + t * 128: dl * CIN + (t + 1) * 128],
                            attn[:], start=True, stop=True, skip_group_check=True,
                        )
                        vv = pad_view(v_b[t], 8 * ci + di, 8, dj + 1)
                        if dl % 2 == 0:
                            # DVE reads psum directly (mixed-dtype 1x), f32r out
                            p2 = workp.tile([128, CHUNK], F32R, tag=f"p2r{t}", name=f"p2r{t}", bufs=5)
                            nc.vector.tensor_mul(p2[:], pe[:], vv)
                            acc_ident = ident_r2
                        else:
                            # ACT casts, gpsimd multiplies (both idle in this phase)
                            eb = workp.tile([128, CHUNK], F32R, tag=f"eb{t}", name=f"eb{t}", bufs=3)
                            nc.scalar.copy(eb[:], pe[:])
                            p2 = workp.tile([128, CHUNK], BF16, tag=f"p2{t}", name=f"p2{t}", bufs=5)
                            nc.gpsimd.tensor_mul(p2[:], eb[:], vv)
                            acc_ident = ident_r
                        nc.tensor.matmul(
                            po[t][:], acc_ident[:], p2[:],
                            start=(dl == 0), stop=(dl == NKK - 1),
                            skip_group_check=True,
                        )
                for t in range(2):
                    ob = workp.tile([128, CHUNK], F32, tag=f"ob{t}", name=f"ob{t}")
                    nc.scalar.copy(ob[:], po[t][:])
                    (nc.sync if t == 0 else nc.scalar).dma_start(
                        outd[t * 128:(t + 1) * 128, ci * CHUNK:(ci + 1) * CHUNK], ob[:]
                    )

            # software pipeline: logits+softmax(ci) overlaps AV(ci-1)
            attn_prev = emit_logits(0)
            for ci in range(1, NCHUNK):
                attn_cur = emit_logits(ci)
                emit_av(ci - 1, attn_prev)
                attn_prev = attn_cur
            emit_av(NCHUNK - 1, attn_prev)

    nc.finalize()
    return nc


_CACHE = {}


def _host_consts(w_proj, b_proj, pos_feats):
    wT = np.ascontiguousarray(w_proj.T).astype(np.float32).copy()   # [256, 768]
    wT[:, : 2 * QK] *= SCALE
    b = np.asarray(b_proj, np.float32).copy()
    b[: 2 * QK] *= SCALE
    bias = np.ascontiguousarray(b.reshape(6, 128).T)                # [128, 6]

    heads = np.arange(CIN) // D                                     # head of channel
    posblk = np.zeros((CIN, NL), np.float32)
    for c in range(CIN):
        n = heads[c]
        posblk[c, n * NKK:(n + 1) * NKK] = pos_feats[c]

    redmat = np.zeros((CIN, NKK * NL), np.float32)
    for dl in range(NKK):
        for c in range(CIN):
            redmat[c, dl * NL + heads[c] * NKK + dl] = 1.0

    sum9 = np.zeros((NL, NH), np.float32)
    e8 = np.zeros((NH, NL), np.float32)
    for n in range(NH):
        for dl in range(NKK):
            sum9[n * NKK + dl, n] = 1.0
            e8[n, n * NKK + dl] = 1.0

    expall = np.zeros((NL, NKK * CIN), np.float32)
    for dl in range(NKK):
        for n in range(NH):
            expall[n * NKK + dl, dl * CIN + n * D: dl * CIN + (n + 1) * D] = 1.0

    import ml_dtypes
    bf = ml_dtypes.bfloat16
    ident = np.eye(128, dtype=np.float32)
    return (wT, bias, posblk.astype(bf), redmat.astype(bf), sum9.astype(bf),
            e8.astype(bf), expall.astype(bf), ident.astype(bf))


def make_in_maps(x, w_proj, b_proj, pos_feats):
    x = np.asarray(x, np.float32)
    wT, bias, posblk, redmat, sum9, e8, expall, ident = _host_consts(
        np.asarray(w_proj, np.float32),
        np.asarray(b_proj, np.float32),
        np.asarray(pos_feats, np.float32),
    )
    in_maps = []
    for s in range(NCORES):
        b_i, half = s // 2, s % 2
        xs = np.zeros((CIN, HS, W), np.float32)
        h0 = half * HOWN - 1                  # global row of local row 0
        lo, hi = max(h0, 0), min(h0 + HS, H)
        xs[:, lo - h0:hi - h0, :] = x[b_i, :, lo:hi, :]
        edge = np.ones((128, 2), np.float32)
        if half == 0:
            edge[:, 0] = 0.0
        if half == 1:
            edge[:, 1] = 0.0
        in_maps.append({
            "x": np.ascontiguousarray(xs.reshape(CIN, PXU)),
            "wt": wT, "bias": bias, "posblk": posblk, "redmat": redmat,
            "sum9": sum9, "e8": e8, "expall": expall, "ident": ident, "edge": edge,
        })
    return in_maps


def kernel(x, w_proj, b_proj, pos_feats):
    from concourse.bass_utils import run_bass_kernel_spmd

    if "nc" not in _CACHE:
        _CACHE["nc"] = _build_nc()
    nc = _CACHE["nc"]
    in_maps = make_in_maps(x, w_proj, b_proj, pos_feats)
    res = run_bass_kernel_spmd(nc, in_maps, list(range(NCORES)))
    out = np.zeros((B, OUT, H, W), np.float32)
    for s in range(NCORES):
        b_i, half = s // 2, s % 2
        out[b_i, :, half * HOWN:(half + 1) * HOWN, :] = (
            res.results[s]["o"].reshape(OUT, HOWN, W)
        )
    return out

